# revision 4
# baseline (speedup 1.0000x reference)
"""Trainium2 Bass kernel for Swin-style window attention (MegatronWindowAttention).

Full computation per window w (49 tokens, dim 256, 8 heads x 32):
  qkv = x @ qkv_w.T + qkv_b ; q,k,v per head
  attn = softmax(q*scale @ k.T + bias + mask[w%64]) ; out = (attn @ v) @ proj_w.T + proj_b

Sharding: data-parallel over the window-batch dim B_=4096 across 8 cores (512 windows each).

Device dataflow per core (4 blocks of 128 windows = 6272 tokens):
  - PE-transpose x tiles -> x.T bf16 (channels on partitions)
  - QKV: W.T-stationary matmuls, tokens moving -> Q.T/K.T/V.T bf16 [oc, t] resident
  - per window-pair (98 tokens): PE-transpose V slices -> V [token, d] padded layout;
    S.T = K.T^T Q.T per (head, window) packed with tile_position; exp on ACT (PSUM->SBUF);
    * exp(bias+mask) multiplicative fold; AV matmuls give O [i, d] + rowsum column
    (ones-augmented V); normalize via DVE reciprocal+mul; PE-transpose O -> O.T;
    proj matmuls; + proj_b; DMA out.
"""

import numpy as np
import ml_dtypes
from contextlib import ExitStack

import concourse.bass as bass
import concourse.tile as tile
import concourse.mybir as mybir
from concourse import bacc
from concourse import bass_utils

WH = WW = 7
NTOK = 49
DIM = 256
NH = 8
HD = 32
SCALE = HD ** -0.5
NCORES = 8
B_FULL = 4096
NW = 64
B_CORE = B_FULL // NCORES          # 512 windows per core
T_CORE = B_CORE * NTOK             # 25088 tokens
NBLOCK = 4
W_BLK = 128                        # windows per block
T_BLK = W_BLK * NTOK               # 6272 tokens per block
T_PAD = T_BLK + 16                 # padded cols so 64-wide window reads stay in-bounds
NPAIR = W_BLK // 2                 # 64 pairs per block
QKV_CHUNK = 448                    # token chunk for qkv matmuls (6272 = 14*448)
NCHUNK = T_BLK // QKV_CHUNK

F32 = mybir.dt.float32
BF16 = mybir.dt.bfloat16
AF = mybir.ActivationFunctionType


def _rel_pos_index():
    coords = np.stack(np.meshgrid(np.arange(WH), np.arange(WW), indexing='ij'))
    flat = coords.reshape(2, -1)
    rel = flat[:, :, None] - flat[:, None, :]
    rel = rel.transpose(1, 2, 0).copy()
    rel[:, :, 0] += WH - 1
    rel[:, :, 1] += WW - 1
    rel[:, :, 0] *= 2 * WW - 1
    return rel.sum(-1).reshape(-1)


def build_kernel(ctx: ExitStack, tc: tile.TileContext, ins: dict, out_ap: bass.AP):
    nc = tc.nc
    x = ins["x"]          # [T_CORE, 256] f32
    qkvwt = ins["qkvwt"]  # [2, 6, 128, 128] bf16
    qkvb = ins["qkvb"]    # [128, 6] f32
    pwt = ins["pwt"]      # [2, 128, 256] bf16
    pbb = ins["pbb"]      # [128, 256] f32
    expb = ins["expb"]    # [32, 113, 392] bf16
    idf = ins["identf"]   # [128, 128] f32
    idb = ins["identb"]   # [128, 128] bf16

    const = ctx.enter_context(tc.tile_pool(name="const", bufs=1))
    qkvwt_sb = const.tile([128, 2 * 6 * 128], BF16, tag="qkvwt")
    nc.sync.dma_start(qkvwt_sb[:], qkvwt.rearrange("p a b c -> p (a b c)"))
    qkvwt_v = qkvwt_sb[:].rearrange("p (a b c) -> p a b c", a=2, b=6)
    qkvb_sb = const.tile([128, 6], F32, tag="qkvb")
    nc.sync.dma_start(qkvb_sb[:], qkvb[:])
    pwt_sb = const.tile([128, 2 * 256], BF16, tag="pwt")
    nc.sync.dma_start(pwt_sb[:], pwt.rearrange("p a c -> p (a c)"))
    pbb_sb = const.tile([128, 256], F32, tag="pbb")
    nc.sync.dma_start(pbb_sb[:], pbb[:])
    expb_sb = const.tile([128, 32 * 392], BF16, tag="expb")
    nc.sync.dma_start(expb_sb[:], expb.rearrange("p m f -> p (m f)"))
    idf_sb = const.tile([128, 128], F32, tag="identf")
    nc.sync.dma_start(idf_sb[:], idf[:])
    idb_sb = const.tile([128, 128], BF16, tag="identb")
    nc.sync.dma_start(idb_sb[:], idb[:])

    xt_pool = ctx.enter_context(tc.tile_pool(name="xt", bufs=1))
    qkv_pool = ctx.enter_context(tc.tile_pool(name="qkvt", bufs=1))
    xin_pool = ctx.enter_context(tc.tile_pool(name="xin", bufs=3))
    v_pool = ctx.enter_context(tc.tile_pool(name="vsb", bufs=2))
    p_pool = ctx.enter_context(tc.tile_pool(name="psb", bufs=2))
    o_pool = ctx.enter_context(tc.tile_pool(name="osb", bufs=2))
    ot_pool = ctx.enter_context(tc.tile_pool(name="otsb", bufs=2))
    r_pool = ctx.enter_context(tc.tile_pool(name="rsb", bufs=2))
    out_pool = ctx.enter_context(tc.tile_pool(name="outsb", bufs=3))

    psA = ctx.enter_context(tc.tile_pool(name="psA", bufs=2, space="PSUM"))
    psS = ctx.enter_context(tc.tile_pool(name="psS", bufs=1, space="PSUM"))
    psW = ctx.enter_context(tc.tile_pool(name="psW", bufs=1, space="PSUM"))

    for blk in range(NBLOCK):
        t0 = blk * T_BLK
        # ---- phase L: load + transpose x -> Xt (bf16, [ic, t]) ----
        Xt = [qkv_pool.tile([128, T_BLK], BF16, tag=f"xt{kb}", name=f"xt{kb}_{blk}") for kb in range(2)]
        for i in range(T_BLK // 128):
            xsb = xin_pool.tile([128, 256], F32, tag="xin")
            nc.sync.dma_start(xsb[:], x[t0 + 128 * i: t0 + 128 * (i + 1), :])
            for kb in range(2):
                tp = psA.tile([128, 448], F32, tag="psA")
                nc.tensor.transpose(tp[:, 0:128], xsb[:, 128 * kb:128 * (kb + 1)], idf_sb[:])
                nc.scalar.copy(Xt[kb][:, 128 * i:128 * (i + 1)], tp[:, 0:128])
        # ---- phase Q: qkv matmuls -> QKVt (bf16, [oc, t]) ----
        QKVt = [qkv_pool.tile([128, T_PAD], BF16, tag=f"qkv{ob}", name=f"qkv{ob}_{blk}") for ob in range(6)]
        for ob in range(6):
            nc.vector.memset(QKVt[ob][:, T_BLK:T_PAD], 0.0)
        for c in range(NCHUNK):
            cs = slice(QKV_CHUNK * c, QKV_CHUNK * (c + 1))
            for ob in range(6):
                qps = psA.tile([128, 448], F32, tag="psA")
                for kb in range(2):
                    nc.tensor.matmul(qps[:], qkvwt_v[:, kb, ob, :], Xt[kb][:, cs],
                                     start=(kb == 0), stop=(kb == 1))
                sc = SCALE if ob in (2, 3) else 1.0
                nc.scalar.activation(QKVt[ob][:, cs], qps[:], AF.Identity,
                                     bias=qkvb_sb[:, ob:ob + 1], scale=sc)
        # ---- phase A: attention per window pair ----
        for u in range(NPAIR):
            c0 = 98 * u
            # V relayout: PE transpose V.T slices -> Vps [t(pad64), oc], evac strided
            vps = psW.tile([128, 256], BF16, tag="psW", name=f"vps_{blk}_{u}")
            for w in range(2):
                for kb in range(2):
                    nc.tensor.transpose(
                        vps[64 * w:64 * w + 64, 128 * kb:128 * (kb + 1)],
                        QKVt[4 + kb][:, c0 + 49 * w: c0 + 49 * w + 64],
                        idb_sb[:], tile_position=(0, 64 * w))
            vsb = v_pool.tile([128, 264], BF16, tag="vsb")
            vsb_v = vsb[:].rearrange("p (h c) -> p h c", h=8)
            vps_v = vps[:].rearrange("p (h c) -> p h c", h=8)
            nc.scalar.copy(vsb_v[:, :, 0:32], vps_v[:, :, :])
            nc.vector.memset(vsb_v[:, :, 32:33], 1.0)
            # S.T matmuls: per (head, window)
            sps = psS.tile([128, 2048], F32, tag="psS")
            for h in range(NH):
                kt = QKVt[2 + h // 4]
                qt = QKVt[0 + h // 4]
                rs = slice(32 * (h % 4), 32 * (h % 4) + 32)
                sc0 = 512 * (h % 4) + 49 * (h // 4)
                for w in range(2):
                    ws64 = slice(c0 + 49 * w, c0 + 49 * w + 64)
                    ws = slice(c0 + 49 * w, c0 + 49 * w + 49)
                    nc.tensor.matmul(
                        sps[64 * w:64 * w + 64, sc0:sc0 + 49],
                        kt[rs, ws64], qt[rs, ws], start=True, stop=True,
                        tile_position=(32 * (h % 4), 64 * w))
            # exp (ACT, PSUM->SBUF) then * exp(bias+mask) (DVE)
            pexp = p_pool.tile([128, 392], BF16, tag="pexp")
            sps_v = sps[:].rearrange("p (b c) -> p b c", b=4)[:, :, 0:98]
            pexp_v = pexp[:].rearrange("p (b c) -> p b c", b=4)
            nc.scalar.activation(pexp_v[:, :, :], sps_v, AF.Exp)
            pm = p_pool.tile([128, 408], BF16, tag="pm")
            nc.vector.tensor_mul(pm[:, 0:392], pexp[:],
                                 expb_sb[:, 392 * (u % 32):392 * (u % 32 + 1)])
            nc.vector.memset(pm[:, 392:408], 1.0)
            # AV: O[i, d] + rowsum col; lhsT = P.T slice, rhs = V(+ones)
            avps = psW.tile([128, 1024], F32, tag="psW", name=f"avps_{blk}_{u}")
            for h in range(NH):
                pc = 98 * (h % 4) + 49 * (h // 4)
                for w in range(2):
                    nc.tensor.matmul(
                        avps[64 * w:64 * w + 64, 512 * w + 33 * h:512 * w + 33 * h + 33],
                        pm[64 * w:64 * w + 49, pc:pc + 64],
                        vsb[64 * w:64 * w + 49, 33 * h:33 * h + 33],
                        start=True, stop=True, tile_position=(64 * w, 64 * w))
            # normalize: recip of rowsum col, broadcast-mult -> Onorm dense [113, 256]
            recip = r_pool.tile([128, 8], F32, tag="recip")
            onorm = o_pool.tile([128, 256], BF16, tag="onorm")
            onorm_v = onorm[:].rearrange("p (h c) -> p h c", h=8)
            for w in range(2):
                pr = slice(64 * w, 64 * w + 64)
                av_w = avps[pr, 512 * w:512 * w + 264].rearrange("p (h c) -> p h c", h=8)
                nc.vector.reciprocal(recip[pr, :], av_w[:, :, 32])
                recip_b = recip[pr, :].unsqueeze(2).broadcast_to([64, 8, 32])
                nc.vector.tensor_mul(onorm_v[pr, :, :], av_w[:, :, 0:32], recip_b)
            # transpose O -> O.T chunks [128 hd, 49] ; cols 49*(2*hb+w)
            otps = psW.tile([128, 2048], BF16, tag="psW", name=f"otps_{blk}_{u}")
            for w in range(2):
                for hb in range(2):
                    nc.tensor.transpose(
                        otps[:, 1024 * w + 512 * hb:1024 * w + 512 * hb + 49],
                        onorm[64 * w:64 * w + 49, 128 * hb:128 * (hb + 1)],
                        idb_sb[64 * w:64 * w + 49, 64 * w:64 * w + 49],
                        tile_position=(64 * w, 0))
            otsb = ot_pool.tile([128, 196], BF16, tag="otsb")
            otps_v = otps[:].rearrange("p (w b c) -> p w b c", w=2, b=2)[:, :, :, 0:49]
            otsb_v = otsb[:].rearrange("p (b w c) -> p w b c", b=2, w=2)
            nc.scalar.copy(otsb_v[:, :, :, :], otps_v)
            # proj: lhsT = O.T block stationary, rhs = proj_w.T moving
            pps = psW.tile([98, 256], F32, tag="psW", name=f"pps_{blk}_{u}")
            for hb in range(2):
                nc.tensor.matmul(pps[:], otsb[:, 98 * hb:98 * (hb + 1)],
                                 pwt_sb[:, 256 * hb:256 * (hb + 1)],
                                 start=(hb == 0), stop=(hb == 1))
            osb = out_pool.tile([98, 256], F32, tag="outsb")
            nc.vector.tensor_add(osb[:], pps[:], pbb_sb[0:98, :])
            nc.sync.dma_start(out_ap[t0 + c0:t0 + c0 + 98, :], osb[:])


_CACHED = {}


def _get_program(niter=1):
    if niter in _CACHED:
        return _CACHED[niter]
    nc = bacc.Bacc("TRN2", target_bir_lowering=False, debug=False)
    ins = {
        "x": nc.dram_tensor("x", [T_CORE, DIM], F32, kind="ExternalInput").ap(),
        "qkvwt": nc.dram_tensor("qkvwt", [128, 2, 6, 128], BF16, kind="ExternalInput").ap(),
        "qkvb": nc.dram_tensor("qkvb", [128, 6], F32, kind="ExternalInput").ap(),
        "pwt": nc.dram_tensor("pwt", [128, 2, 256], BF16, kind="ExternalInput").ap(),
        "pbb": nc.dram_tensor("pbb", [128, 256], F32, kind="ExternalInput").ap(),
        "expb": nc.dram_tensor("expb", [128, 32, 392], BF16, kind="ExternalInput").ap(),
        "identf": nc.dram_tensor("identf", [128, 128], F32, kind="ExternalInput").ap(),
        "identb": nc.dram_tensor("identb", [128, 128], BF16, kind="ExternalInput").ap(),
    }
    out_ap = nc.dram_tensor("out", [T_CORE, DIM], F32, kind="ExternalOutput").ap()
    with tile.TileContext(nc) as tc:
        for _ in range(niter):
            with ExitStack() as ctx:
                build_kernel(ctx, tc, ins, out_ap)
    nc.compile()
    _CACHED[niter] = nc
    return nc


def _host_prep(mask, qkv_w, qkv_b, proj_w, proj_b, bias_table):
    bf = ml_dtypes.bfloat16
    qkvwt = np.ascontiguousarray(
        qkv_w.reshape(6, 128, 2, 128).transpose(3, 2, 0, 1)).astype(bf)
    qb = np.asarray(qkv_b, np.float32).copy()
    qb[256:512] *= SCALE
    qkvb = np.ascontiguousarray(qb.reshape(6, 128).T)
    pwt = np.ascontiguousarray(np.asarray(proj_w, np.float32).T.reshape(2, 128, 256).transpose(1, 0, 2)).astype(bf)
    pbb = np.ascontiguousarray(np.broadcast_to(np.asarray(proj_b, np.float32), (128, 256)))
    # combined exp(bias + mask), transposed to [j, i], packed per pair pattern
    rel = _rel_pos_index()
    bias_g = np.asarray(bias_table, np.float32)[rel].reshape(NTOK, NTOK, NH)  # [i, j, h]
    comb = bias_g[None].transpose(0, 3, 1, 2) + np.asarray(mask, np.float32)[:, None]  # [64, h, i, j]
    combT = np.exp(comb.transpose(0, 1, 3, 2))  # [64, h, j, i]
    expb = np.zeros((32, 128, 392), np.float32)
    for p in range(32):
        for h in range(NH):
            hc = 98 * (h % 4) + 49 * (h // 4)
            expb[p, 0:49, hc:hc + 49] = combT[2 * p, h]
            expb[p, 64:113, hc:hc + 49] = combT[2 * p + 1, h]
    expb = np.ascontiguousarray(expb.transpose(1, 0, 2)).astype(bf)
    identf = np.eye(128, dtype=np.float32)
    identb = np.eye(128).astype(bf)
    return qkvwt, qkvb, pwt, pbb, expb, identf, identb


def _make_in_maps(x, prep):
    qkvwt, qkvb, pwt, pbb, expb, identf, identb = prep
    xs = x.reshape(B_FULL, NTOK, DIM)
    in_maps = []
    for c in range(NCORES):
        shard = np.ascontiguousarray(
            xs[c * B_CORE:(c + 1) * B_CORE].reshape(T_CORE, DIM))
        in_maps.append({
            "x": shard, "qkvwt": qkvwt, "qkvb": qkvb, "pwt": pwt, "pbb": pbb,
            "expb": expb, "identf": identf, "identb": identb,
        })
    return in_maps


def kernel(x, mask, qkv_w, qkv_b, proj_w, proj_b, bias_table, _trace=False):
    x = np.asarray(x, np.float32)
    prep = _host_prep(
        np.asarray(mask), np.asarray(qkv_w), np.asarray(qkv_b),
        np.asarray(proj_w), np.asarray(proj_b), np.asarray(bias_table))
    in_maps = _make_in_maps(x, prep)
    nc = _get_program()
    res = bass_utils.run_bass_kernel_spmd(nc, in_maps, core_ids=list(range(NCORES)),
                                          trace=_trace)
    out = np.stack([r["out"] for r in res.results])  # [8, T_CORE, 256]
    out = out.reshape(B_FULL, NTOK, DIM)
    if _trace:
        kernel.last_results = res
    return out



# revision 20
# speedup vs baseline: 2.0366x; 2.0366x over previous
"""Trainium2 Bass kernel for Swin-style window attention (MegatronWindowAttention).

Per window w (49 tokens, dim 256, 8 heads x 32):
  qkv = x @ qkv_w.T + qkv_b ; q,k,v per head
  attn = softmax(q*scale @ k.T + bias + mask[w%64]) ; out = (attn @ v) @ proj_w.T + proj_b

Sharding: data-parallel over the window-batch dim B_=4096 across 8 cores (512
windows each).

v2 design (vs v1): x.T is pre-transposed to bf16 on the host (no on-device
transpose phase), 8 blocks of 64 windows double-buffered so blocks pipeline,
PSUM tiles are bank-tight with bufs=2 so window-pairs pipeline across engines,
attention matmuls use 49-col stationaries, elementwise ops are merged over
2-pair groups and split across ACT/DVE/GPSIMD, output DMA in bf16.

Device dataflow per core, per block of 64 windows (3136 tokens):
  - DMA x.T bf16 -> Xt[2] resident
  - QKV: W.T-stationary matmuls (2 obs per PSUM tile), evac via ACT
    (identity+bias, q/k) and DVE (tensor-add bias, v) -> Q.T/K.T/V.T bf16
  - per pair (2 windows): PE-transpose V.T -> vsb [tok, hd] + ones col;
    S.T = K.T^T Q.T per (head, window), 8 head-blocks packed in one PSUM bank;
    exp on ACT per 2-pair group; pm = pexp*exp(bias+mask) on GPSIMD;
    AV -> O [i, hd]+rowsum; DVE recip+normalize (both windows in one op);
    PE-transpose O -> O.T; proj matmuls; +proj_b on DVE (doubles as evac);
    bf16 DMA out per group.
"""

import os
import numpy as np
import ml_dtypes
from contextlib import ExitStack

import concourse.bass as bass
import concourse.tile as tile
import concourse.mybir as mybir
from concourse import bacc
from concourse import bass_utils

WH = WW = 7
NTOK = 49
DIM = 256
NH = 8
HD = 32
SCALE = HD ** -0.5
NCORES = 8
B_FULL = 4096
NW = 64
B_CORE = B_FULL // NCORES          # 512 windows per core
T_CORE = B_CORE * NTOK             # 25088 tokens
NBLOCK = 8
W_BLK = 64                         # windows per block
T_BLK = W_BLK * NTOK               # 3136 tokens per block
NPAIR = W_BLK // 2                 # 32 pairs per block
NSG = NPAIR // 4                   # 8 supergroups (4 pairs) per block
T_PAD = T_BLK + 16                 # padded cols so 64-wide window reads stay in-bounds
QKV_CHUNK = 448                    # token chunk for qkv matmuls (3136 = 7*448)
NCHUNK = T_BLK // QKV_CHUNK

F32 = mybir.dt.float32
BF16 = mybir.dt.bfloat16
AF = mybir.ActivationFunctionType


def _rel_pos_index():
    coords = np.stack(np.meshgrid(np.arange(WH), np.arange(WW), indexing='ij'))
    flat = coords.reshape(2, -1)
    rel = flat[:, :, None] - flat[:, None, :]
    rel = rel.transpose(1, 2, 0).copy()
    rel[:, :, 0] += WH - 1
    rel[:, :, 1] += WW - 1
    rel[:, :, 0] *= 2 * WW - 1
    return rel.sum(-1).reshape(-1)


def build_kernel(ctx: ExitStack, tc: tile.TileContext, ins: dict, out_ap: bass.AP):
    nc = tc.nc
    sim_init = bool(os.environ.get("SIM_INIT"))
    xt = ins["xt"]        # [2, 128, T_CORE] bf16 (pre-transposed x)
    qkvwt = ins["qkvwt"]  # [2, 6, 128, 128] bf16 (k-scale folded)
    qkvb = ins["qkvb"]    # [128, 6] f32 (k-scale folded)
    pwt = ins["pwt"]      # [2, 128, 256] bf16
    pbb = ins["pbb"]      # [128, 256] f32
    expb = ins["expb"]    # [128, NSG, 1568] bf16 (per 4-pair supergroup)
    idb = ins["identb"]   # [128, 128] bf16

    const = ctx.enter_context(tc.tile_pool(name="const", bufs=1))
    qkvwt_sb = const.tile([128, 2 * 6 * 128], BF16, tag="qkvwt")
    nc.sync.dma_start(qkvwt_sb[:], qkvwt.rearrange("p a b c -> p (a b c)"))
    qkvwt_v = qkvwt_sb[:].rearrange("p (a b c) -> p a b c", a=2, b=6)
    qkvb_sb = const.tile([128, 6], F32, tag="qkvb")
    nc.sync.dma_start(qkvb_sb[:], qkvb[:])
    pwt_sb = const.tile([128, 2 * 256], BF16, tag="pwt")
    nc.sync.dma_start(pwt_sb[:], pwt.rearrange("p a c -> p (a c)"))
    pbb_sb = const.tile([128, 256], F32, tag="pbb")
    nc.sync.dma_start(pbb_sb[:], pbb[:])
    expb_sb = const.tile([128, NSG * 1568], BF16, tag="expb")
    nc.sync.dma_start(expb_sb[:], expb.rearrange("p m f -> p (m f)"))
    idb_sb = const.tile([128, 128], BF16, tag="identb")
    nc.sync.dma_start(idb_sb[:], idb[:])

    qkv_pool = ctx.enter_context(tc.tile_pool(name="qkvt", bufs=2))
    v_pool = ctx.enter_context(tc.tile_pool(name="vsb", bufs=2))
    p_pool = ctx.enter_context(tc.tile_pool(name="psb", bufs=2))
    o_pool = ctx.enter_context(tc.tile_pool(name="osb", bufs=2))
    ot_pool = ctx.enter_context(tc.tile_pool(name="otsb", bufs=3))
    r_pool = ctx.enter_context(tc.tile_pool(name="rsb", bufs=3))
    out_pool = ctx.enter_context(tc.tile_pool(name="outsb", bufs=3))

    for blk in range(NBLOCK):
        t0 = blk * T_BLK
        Xt = [qkv_pool.tile([128, T_BLK], BF16, tag=f"xt{kb}", name=f"xt{kb}_{blk}")
              for kb in range(2)]
        for kb in range(2):
            nc.sync.dma_start(Xt[kb][:], xt[kb, :, t0:t0 + T_BLK])
        QKVt = [qkv_pool.tile([128, T_PAD], BF16, tag=f"qkv{ob}", name=f"qkv{ob}_{blk}")
                for ob in range(6)]
        for ob in range(2, 6):  # k/v are read with 64-wide window slices
            nc.vector.memset(QKVt[ob][:, T_BLK:T_PAD], 0.0)
        vsbb = v_pool.tile([128, 264 * NPAIR], BF16, tag="vsbb", name=f"vsbb_{blk}")
        vsbb_v = vsbb[:].rearrange("p (u h c) -> p u h c", u=NPAIR, h=8)

        # ---- phase Q: qkv matmuls + V relayout (own PSUM scope) ----
        with tc.tile_pool(name=f"psq{blk}", bufs=2, space="PSUM") as ps_q, \
             tc.tile_pool(name=f"psvq{blk}", bufs=2, space="PSUM") as ps_vq:
            for c in range(NCHUNK):
                cs = slice(QKV_CHUNK * c, QKV_CHUNK * (c + 1))
                for op in range(3):  # ob pairs: (0,1)=q, (2,3)=k, (4,5)=v
                    qps = ps_q.tile([128, 1024], F32, tag="q",
                                    name=f"qps_{blk}_{c}_{op}")
                    for half in range(2):
                        ob = 2 * op + half
                        hs = slice(512 * half, 512 * half + 448)
                        for kb in range(2):
                            nc.tensor.matmul(qps[:, hs], qkvwt_v[:, kb, ob, :],
                                             Xt[kb][:, cs],
                                             start=(kb == 0), stop=(kb == 1))
                    qps_v = qps[:].rearrange("p (h c) -> p h c", h=2)[:, :, 0:448]
                    dst = [QKVt[2 * op][:, cs], QKVt[2 * op + 1][:, cs]]
                    if op < 2:
                        for half in range(2):
                            nc.scalar.activation(
                                dst[half], qps_v[:, half, :], AF.Identity,
                                bias=qkvb_sb[:, 2 * op + half:2 * op + half + 1])
                    else:
                        for half in range(2):
                            bb = qkvb_sb[:, 4 + half:5 + half].broadcast_to(
                                [128, QKV_CHUNK])
                            nc.vector.tensor_add(dst[half], qps_v[:, half, :], bb)
            # V relayout for all pairs of the block
            for u in range(NPAIR):
                c0 = 98 * u
                vps = ps_vq.tile([128, 256], BF16, tag="v", name=f"vps_{blk}_{u}")
                for w in range(2):
                    for kb in range(2):
                        nc.tensor.transpose(
                            vps[64 * w:64 * w + 64, 128 * kb:128 * (kb + 1)],
                            QKVt[4 + kb][:, c0 + 49 * w: c0 + 49 * w + 64],
                            idb_sb[:], tile_position=(0, 64 * w))
                vps_v = vps[:].rearrange("p (h c) -> p h c", h=8)
                nc.scalar.copy(vsbb_v[:, u, :, 0:32], vps_v[:, :, :])
                nc.vector.memset(vsbb_v[:, u, :, 32:33], 1.0)

        # ---- phase A: attention per 4-pair supergroup (own PSUM scope) ----
        with tc.tile_pool(name=f"pss{blk}", bufs=1, space="PSUM") as ps_s, \
             tc.tile_pool(name=f"psw{blk}", bufs=2, space="PSUM") as ps_w:
            for sg in range(NSG):
                spsg = ps_s.tile([128, 2048], F32, tag="s", name=f"sps_{blk}_{sg}")
                # S.T: bank = h%4 (row strip), block col 49*(2*u4 + h//4)
                for u4 in range(4):
                    c0 = 98 * (4 * sg + u4)
                    for h in range(NH):
                        kt = QKVt[2 + h // 4]
                        qt = QKVt[0 + h // 4]
                        rs = slice(32 * (h % 4), 32 * (h % 4) + 32)
                        sc = 512 * (h % 4) + 49 * (2 * u4 + h // 4)
                        for w in range(2):
                            ws64 = slice(c0 + 49 * w, c0 + 49 * w + 64)
                            ws = slice(c0 + 49 * w, c0 + 49 * w + 49)
                            nc.tensor.matmul(
                                spsg[64 * w:64 * w + 64, sc:sc + 49],
                                kt[rs, ws64], qt[rs, ws], start=True, stop=True,
                                tile_position=(32 * (h % 4), 64 * w))
                pexp = p_pool.tile([128, 1568], BF16, tag="pexp")
                spsg_v = spsg[:].rearrange("p (r c) -> p r c", r=4)[:, :, 0:392]
                pexp_v = pexp[:].rearrange("p (r c) -> p r c", r=4)
                nc.scalar.activation(pexp_v[:, :, :], spsg_v, AF.Exp)
                pm = p_pool.tile([128, 1584], BF16, tag="pm")
                nc.vector.tensor_mul(pm[:, 0:1568], pexp[:],
                                     expb_sb[:, 1568 * sg:1568 * (sg + 1)])
                nc.vector.memset(pm[:, 1568:1584], 1.0)
                for u4 in range(4):
                    u = 4 * sg + u4
                    # AV: O[i, d]+rowsum; banks split by w (row strip = 64w)
                    avps = ps_w.tile([128, 1024], F32, tag="w", name=f"av_{blk}_{u}")
                    if sim_init:
                        nc.vector.memset(avps[0:64, 512:776], 1.0)
                        nc.vector.memset(avps[64:128, 0:264], 1.0)
                    for h in range(NH):
                        pc = 392 * (h % 4) + 49 * (2 * u4 + h // 4)
                        for w in range(2):
                            nc.tensor.matmul(
                                avps[64 * w:64 * w + 64,
                                     512 * w + 33 * h:512 * w + 33 * h + 33],
                                pm[64 * w:64 * w + 49, pc:pc + 64],
                                vsbb[64 * w:64 * w + 49,
                                     264 * u + 33 * h:264 * u + 33 * h + 33],
                                start=True, stop=True,
                                tile_position=(64 * w, 64 * w))
                    # normalize both windows in one recip + one mul
                    recip = r_pool.tile([128, 16], F32, tag="recip")
                    avps_v = avps[:].rearrange("p (v c) -> p v c", v=2)[:, :, 0:264] \
                        .rearrange("p v (h c) -> p v h c", h=8)
                    nc.vector.reciprocal(recip[:].rearrange("p (v h) -> p v h", v=2),
                                         avps_v[:, :, :, 32])
                    recip_b = recip[:].rearrange("p (v h) -> p v h", v=2)\
                        .unsqueeze(3).broadcast_to([128, 2, 8, 32])
                    onorm = o_pool.tile([128, 1024], BF16, tag="onorm")
                    onorm_v = onorm[:].rearrange("p (v c) -> p v c", v=2)[:, :, 0:256] \
                        .rearrange("p v (h c) -> p v h c", h=8)
                    nc.vector.tensor_mul(onorm_v, avps_v[:, :, :, 0:32], recip_b)
                    # O.T transposes: banks split by w; 50-col slots (alignment)
                    otps = ps_w.tile([128, 2048], BF16, tag="w", name=f"ot_{blk}_{u}")
                    for w in range(2):
                        for hb in range(2):
                            oc0 = 1024 * w + 100 * hb
                            nc.tensor.transpose(
                                otps[:, oc0:oc0 + 49],
                                onorm[64 * w:64 * w + 49,
                                      512 * w + 128 * hb:512 * w + 128 * (hb + 1)],
                                idb_sb[64 * w:64 * w + 49, 64 * w:64 * w + 49],
                                tile_position=(64 * w, 0))
                    otsb = ot_pool.tile([128, 196], BF16, tag="otsb")
                    otps_v = otps[:].rearrange("p (v c) -> p v c", v=2)[:, :, 0:200] \
                        .rearrange("p v (b c) -> p v b c", b=2)[:, :, :, 0:49]
                    otsb_v = otsb[:].rearrange("p (b v c) -> p v b c", b=2, v=2)
                    nc.scalar.copy(otsb_v[:, :, :, :], otps_v)
                    pps = ps_w.tile([98, 256], F32, tag="w", name=f"pp_{blk}_{u}")
                    for hb in range(2):
                        nc.tensor.matmul(pps[:], otsb[:, 98 * hb:98 * (hb + 1)],
                                         pwt_sb[:, 256 * hb:256 * (hb + 1)],
                                         start=(hb == 0), stop=(hb == 1))
                    osb = out_pool.tile([98, 256], BF16, tag="outsb")
                    nc.vector.tensor_add(osb[:], pps[:], pbb_sb[0:98, :])
                    nc.sync.dma_start(out_ap[t0 + 98 * u:t0 + 98 * u + 98, :], osb[:])


_CACHED = {}


def _get_program(niter=1):
    if niter in _CACHED:
        return _CACHED[niter]
    nc = bacc.Bacc("TRN2", target_bir_lowering=False, debug=False)
    ins = {
        "xt": nc.dram_tensor("xt", [2, 128, T_CORE], BF16, kind="ExternalInput").ap(),
        "qkvwt": nc.dram_tensor("qkvwt", [128, 2, 6, 128], BF16, kind="ExternalInput").ap(),
        "qkvb": nc.dram_tensor("qkvb", [128, 6], F32, kind="ExternalInput").ap(),
        "pwt": nc.dram_tensor("pwt", [128, 2, 256], BF16, kind="ExternalInput").ap(),
        "pbb": nc.dram_tensor("pbb", [128, 256], F32, kind="ExternalInput").ap(),
        "expb": nc.dram_tensor("expb", [128, 8, 1568], BF16, kind="ExternalInput").ap(),
        "identb": nc.dram_tensor("identb", [128, 128], BF16, kind="ExternalInput").ap(),
    }
    out_ap = nc.dram_tensor("out", [T_CORE, DIM], BF16, kind="ExternalOutput").ap()
    with tile.TileContext(nc) as tc:
        for _ in range(niter):
            with ExitStack() as ctx:
                build_kernel(ctx, tc, ins, out_ap)
    nc.compile()
    _CACHED[niter] = nc
    return nc


def _host_prep(mask, qkv_w, qkv_b, proj_w, proj_b, bias_table):
    bf = ml_dtypes.bfloat16
    qw = np.asarray(qkv_w, np.float32).copy()
    qw[256:512] *= SCALE                      # fold attention scale into k
    qkvwt = np.ascontiguousarray(
        qw.reshape(6, 128, 2, 128).transpose(3, 2, 0, 1)).astype(bf)
    qb = np.asarray(qkv_b, np.float32).copy()
    qb[256:512] *= SCALE
    qkvb = np.ascontiguousarray(qb.reshape(6, 128).T)
    pwt = np.ascontiguousarray(
        np.asarray(proj_w, np.float32).T.reshape(2, 128, 256).transpose(1, 0, 2)).astype(bf)
    pbb = np.ascontiguousarray(
        np.broadcast_to(np.asarray(proj_b, np.float32), (128, 256)))
    # combined exp(bias + mask), transposed to [j, i], packed per 2-pair group
    rel = _rel_pos_index()
    bias_g = np.asarray(bias_table, np.float32)[rel].reshape(NTOK, NTOK, NH)  # [i,j,h]
    comb = bias_g[None].transpose(0, 3, 1, 2) + np.asarray(mask, np.float32)[:, None]
    combT = np.exp(comb.transpose(0, 1, 3, 2))  # [64, h, j, i]
    expb = np.zeros((8, 128, 1568), np.float32)
    for sg in range(8):
        for u4 in range(4):
            p = 4 * sg + u4
            for h in range(NH):
                hc = 392 * (h % 4) + 49 * (2 * u4 + h // 4)
                expb[sg, 0:49, hc:hc + 49] = combT[2 * p, h]
                expb[sg, 64:113, hc:hc + 49] = combT[2 * p + 1, h]
    expb = np.ascontiguousarray(expb.transpose(1, 0, 2)).astype(bf)
    identb = np.eye(128).astype(bf)
    return qkvwt, qkvb, pwt, pbb, expb, identb


def _make_in_maps(x, prep):
    qkvwt, qkvb, pwt, pbb, expb, identb = prep
    bf = ml_dtypes.bfloat16
    xb = np.asarray(x, np.float32).astype(bf).reshape(B_FULL, NTOK, DIM)
    in_maps = []
    for c in range(NCORES):
        shard = xb[c * B_CORE:(c + 1) * B_CORE].reshape(T_CORE, 2, 128)
        xtc = np.ascontiguousarray(shard.transpose(1, 2, 0))   # [2, 128, T_CORE]
        in_maps.append({
            "xt": xtc, "qkvwt": qkvwt, "qkvb": qkvb, "pwt": pwt, "pbb": pbb,
            "expb": expb, "identb": identb,
        })
    return in_maps


def kernel(x, mask, qkv_w, qkv_b, proj_w, proj_b, bias_table, _trace=False):
    prep = _host_prep(
        np.asarray(mask), np.asarray(qkv_w), np.asarray(qkv_b),
        np.asarray(proj_w), np.asarray(proj_b), np.asarray(bias_table))
    in_maps = _make_in_maps(np.asarray(x), prep)
    nc = _get_program()
    res = bass_utils.run_bass_kernel_spmd(nc, in_maps, core_ids=list(range(NCORES)),
                                          trace=_trace)
    out = np.stack([np.asarray(r["out"], np.float32) for r in res.results])
    out = out.reshape(B_FULL, NTOK, DIM)
    if _trace:
        kernel.last_results = res
    return out
